# revision 1
# baseline (speedup 1.0000x reference)
"""Causal self-attention (B=4, T=2048, C=1024, 16 heads, rope) on 8 trn2
NeuronCores, tensor-parallel over heads (2 heads/core).

Each core gets the full token stream plus its head-group's W_attn columns /
W_proj rows, computes a full-shape partial of the output projection, and the
host sums the 8 partials (the all-reduce) and transposes back.

All matmuls run as float32r (full PE rate, ~1e-4 rel err). Scores are
computed transposed ([k, q] layout) so softmax(P) @ V needs no transposes;
the softmax denominator comes from an extra ones-stationary matmul whose
M=64 output is already broadcast across partitions.
"""

import ml_dtypes
import numpy as np

import concourse.bacc as bacc
import concourse.mybir as mybir
import concourse.tile as tile
from concourse.bass_utils import run_bass_kernel_spmd

F32 = mybir.dt.float32
F32R = mybir.dt.float32r
BF16 = mybir.dt.bfloat16
AF = mybir.ActivationFunctionType

B, T, C = 4, 2048, 1024
N_HEAD, HEAD_DIM = 16, 64
N_CORES = 8
HPC = N_HEAD // N_CORES          # heads per core = 2
HF = HPC * HEAD_DIM              # per-core head features = 128
NT = B * T                       # 8192 tokens
KT = C // 128                    # 8 contraction tiles for qkv proj
QC = 512                         # query-chunk width
NQC = T // QC                    # 4 query chunks per batch
ROPE_BASE = 10000.0
SCALE = 1.0 / 8.0                # 1/sqrt(HEAD_DIM)

_PROGRAM = None


def _patch_act_tables():
    """Make Exp/Ln resolve only to the combined natural_log_exp set so the
    table-load pass doesn't thrash between exp-only and ln-only sets.
    Set order (and thus act_func_set_id indices) is preserved."""
    import concourse.bacc as _bacc_mod
    from concourse import hw_specs as _hw

    if getattr(_bacc_mod, "_act_tables_patched", False):
        return
    _orig = _hw.get_activation_tables

    def _patched(arch):
        tabs = {k: set(v) for k, v in _orig(arch).items()}
        if "natural_log_exp_and_others" in tabs:
            for name, fns in tabs.items():
                if name != "natural_log_exp_and_others":
                    fns.discard(AF.Exp)
                    fns.discard(AF.Ln)
        return tabs

    _bacc_mod.get_activation_tables = _patched
    _bacc_mod._act_tables_patched = True


def _build_program():
    _patch_act_tables()
    nc = bacc.Bacc(None, target_bir_lowering=False)

    xT = nc.dram_tensor("xT", [C, NT], F32R, kind="ExternalInput")
    wq = nc.dram_tensor("wq", [C, HF], F32R, kind="ExternalInput")
    wk = nc.dram_tensor("wk", [C, HF], F32R, kind="ExternalInput")
    wv = nc.dram_tensor("wv", [C, HF], F32R, kind="ExternalInput")
    wp = nc.dram_tensor("wp", [HF, C], F32R, kind="ExternalInput")
    identd = nc.dram_tensor("identd", [128, 128], F32R, kind="ExternalInput")
    onesdd = nc.dram_tensor("onesdd", [128, 64], F32R, kind="ExternalInput")
    cosd = nc.dram_tensor("cosd", [HF, T], F32R, kind="ExternalInput")
    ssind = nc.dram_tensor("ssind", [HF, T], F32R, kind="ExternalInput")
    outT = nc.dram_tensor("outT", [C, NT], F32, kind="ExternalOutput")

    with tile.TileContext(nc) as tc:
        with (
            tc.tile_pool(name="const", bufs=1) as cpool,
            tc.tile_pool(name="sx", bufs=12) as sx,
            tc.tile_pool(name="srope", bufs=2) as srope,
            tc.tile_pool(name="svt", bufs=4) as svt,
            tc.tile_pool(name="schunk", bufs=2) as schunk,
            tc.tile_pool(name="sv", bufs=2) as sv,
            tc.tile_pool(name="spt", bufs=4) as spt,
            tc.tile_pool(name="sy", bufs=2) as sy,
            tc.tile_pool(name="sst", bufs=2) as sst,
            tc.tile_pool(name="pmm", bufs=2, space="PSUM") as pmm,
            tc.tile_pool(name="psc", bufs=2, space="PSUM") as psc,
            tc.tile_pool(name="py", bufs=2, space="PSUM") as py,
        ):
            # ---- constants (DMA order = first-use order) ----
            wqs = cpool.tile([128, C], F32R, tag="wqs")
            wks = cpool.tile([128, C], F32R, tag="wks")
            wvs = cpool.tile([128, C], F32R, tag="wvs")
            for kt in range(KT):
                nc.gpsimd.dma_start(wqs[:, kt * HF:(kt + 1) * HF], wq[kt * 128:(kt + 1) * 128, :])
                nc.gpsimd.dma_start(wks[:, kt * HF:(kt + 1) * HF], wk[kt * 128:(kt + 1) * 128, :])
                nc.gpsimd.dma_start(wvs[:, kt * HF:(kt + 1) * HF], wv[kt * 128:(kt + 1) * 128, :])
            cost = cpool.tile([128, T], F32R, tag="cost")
            nc.gpsimd.dma_start(cost[:], cosd[:])
            ssint = cpool.tile([128, T], F32R, tag="ssint")
            nc.gpsimd.dma_start(ssint[:], ssind[:])
            ident = cpool.tile([128, 128], F32R, tag="ident")
            nc.gpsimd.dma_start(ident[:], identd[:])
            onesd = cpool.tile([128, 64], F32R, tag="onesd")
            nc.gpsimd.dma_start(onesd[:], onesdd[:])
            wps = cpool.tile([128, C], F32R, tag="wps")
            nc.gpsimd.dma_start(wps[:], wp[:])

            # per-batch state produced by the front end, consumed by attention
            state = {}

            def frontend(b, c):
                """QKV + rope + v-transpose for 512-token chunk c of batch b."""
                if c == 0:
                    state[b] = {
                        "rope_q": srope.tile([128, T], F32R, tag="rope_q", name=f"ropeq{b}"),
                        "rope_k": srope.tile([128, T], F32R, tag="rope_k", name=f"ropek{b}"),
                        "vv": sv.tile([128, (T // 128) * 130], F32R, tag="vv", name=f"vv{b}"),
                    }
                    nc.sync.dma_start(
                        state[b]["vv"][:, 64:(T // 128) * 130:65], onesd[:, 0:32])
                st = state[b]
                boff = b * T
                cc = slice(c * QC, (c + 1) * QC)
                xs = [sx.tile([128, QC], F32R, tag="xs", name=f"xs{b}_{c}_{kt}")
                      for kt in range(KT)]
                for kt in range(KT):
                    nc.sync.dma_start(
                        xs[kt][:],
                        xT[kt * 128:(kt + 1) * 128, boff + c * QC: boff + (c + 1) * QC],
                    )
                vtc = svt.tile([128, QC], F32R, tag="vtc", name=f"vt{b}_{c}")
                for wslab, kind in ((wqs, "q"), (wks, "k"), (wvs, "v")):
                    ps = pmm.tile([128, QC], F32, tag="mm", name=f"qkv{b}_{c}_{kind}")
                    for kt in range(KT):
                        nc.tensor.matmul(
                            ps[:], wslab[:, kt * HF:(kt + 1) * HF], xs[kt][:],
                            start=(kt == 0), stop=(kt == KT - 1),
                        )
                    if kind == "v":
                        nc.vector.tensor_copy(vtc[:], ps[:])
                        continue
                    dst = st["rope_q"] if kind == "q" else st["rope_k"]
                    raw = schunk.tile([128, QC], F32R, tag="rawc", name=f"raw{b}_{c}_{kind}")
                    ta = schunk.tile([128, QC], F32R, tag="tac", name=f"ta{b}_{c}_{kind}")
                    nc.vector.tensor_copy(raw[:], ps[:])
                    nc.vector.tensor_mul(ta[:], ps[:], cost[:, cc])
                    shuf = schunk.tile([128, QC], F32R, tag="shufc", name=f"sh{b}_{c}_{kind}")
                    for h in range(HPC):
                        hb = h * 64
                        nc.gpsimd.dma_start(shuf[hb:hb + 32, :], raw[hb + 32:hb + 64, :])
                        nc.gpsimd.dma_start(shuf[hb + 32:hb + 64, :], raw[hb:hb + 32, :])
                    tb = schunk.tile([128, QC], F32R, tag="tbc", name=f"tb{b}_{c}_{kind}")
                    nc.vector.tensor_mul(tb[:], shuf[:], ssint[:, cc])
                    nc.gpsimd.tensor_add(dst[:, cc], ta[:], tb[:])
                # v chunk -> natural layout [tok, feat] with ones cols
                vv = st["vv"]
                for ki in range(4 * c, 4 * c + 4):
                    pst = pmm.tile([128, QC], F32R, tag="mm", name=f"vt{b}_{ki}")
                    nc.tensor.transpose(
                        pst[:, 0:128], vtc[:, (ki - 4 * c) * 128:(ki - 4 * c + 1) * 128],
                        ident[:])
                    nc.vector.tensor_copy(vv[:, ki * 130:ki * 130 + 64], pst[:, 0:64])
                    nc.vector.tensor_copy(vv[:, ki * 130 + 65:ki * 130 + 129], pst[:, 64:128])

            def attention_chunk(b, qc):
                """Scores/softmax/PV + normalize + projection for query chunk."""
                st = state[b]
                rope_q, rope_k, vv = st["rope_q"], st["rope_k"], st["vv"]
                boff = b * T
                qs = qc * QC
                nki = (qs + QC) // 128
                y_aug = [py.tile([65, QC], F32, tag="y", name=f"y{b}_{qc}_{h}")
                         for h in range(HPC)]
                for ki in range(nki):
                    c0 = max(0, ki * 128 - qs)
                    diag = ki * 128 >= qs
                    sc = psc.tile([128, 2 * QC], F32, tag="sc", name=f"sc{b}_{qc}_{ki}")
                    for h in range(HPC):
                        hb = h * 64
                        nc.tensor.matmul(
                            sc[:, h * QC + c0:(h + 1) * QC],
                            rope_k[hb:hb + 64, ki * 128:(ki + 1) * 128],
                            rope_q[hb:hb + 64, qs + c0:qs + QC],
                            start=True, stop=True,
                        )
                    pt = spt.tile([128, 2 * QC], F32R, tag="pt", name=f"pt{b}_{qc}_{ki}")
                    nc.scalar.activation(
                        pt[:, c0:2 * QC], sc[:, c0:2 * QC], AF.Exp, scale=SCALE,
                    )
                    for h in range(HPC):
                        if diag:  # zero where k > q
                            nc.gpsimd.affine_select(
                                pt[:, h * QC + c0: h * QC + c0 + 128],
                                pt[:, h * QC + c0: h * QC + c0 + 128],
                                pattern=[[1, 128]],
                                compare_op=mybir.AluOpType.is_ge,
                                fill=0.0, base=0, channel_multiplier=-1,
                            )
                        nc.tensor.matmul(
                            y_aug[h][0:65, c0:QC],
                            vv[:, ki * 130 + 65 * h: ki * 130 + 65 * h + 65],
                            pt[:, h * QC + c0: (h + 1) * QC],
                            start=(ki == 0),
                            stop=(ki == nki - 1),
                        )
                ynorm = sy.tile([128, QC], F32R, tag="ynorm", name=f"yn{b}_{qc}")
                for h in range(HPC):
                    lnt = sst.tile([128, QC], F32R, tag="lnt")
                    nc.scalar.activation(lnt[64:65, :], y_aug[h][64:65, :], AF.Ln)
                    rec = sst.tile([128, QC], F32R, tag="rec")
                    nc.scalar.activation(rec[64:65, :], lnt[64:65, :], AF.Exp, scale=-1.0)
                    bc = pmm.tile([128, QC], F32, tag="mm", name=f"bc{b}_{qc}_{h}")
                    nc.tensor.matmul(
                        bc[0:64, :], onesd[64:65, 0:64], rec[64:65, :],
                        start=True, stop=True,
                    )
                    bcs = sst.tile([64, QC], F32, tag="bcs")
                    nc.vector.tensor_copy(bcs[:], bc[0:64, :])
                    if h == 0:
                        nc.vector.tensor_mul(ynorm[0:64, :], y_aug[h][0:64, :], bcs[:])
                    else:
                        hn = sst.tile([64, QC], F32R, tag="hn")
                        nc.vector.tensor_mul(hn[:], y_aug[h][0:64, :], bcs[:])
                        nc.sync.dma_start(ynorm[64:128, :], hn[:])
                # ---- output projection for this chunk (out^T partial) ----
                for of in range(KT):
                    op = pmm.tile([128, QC], F32, tag="mm", name=f"op{b}_{qc}_{of}")
                    nc.tensor.matmul(
                        op[:], wps[:, of * 128:(of + 1) * 128], ynorm[:],
                        start=True, stop=True,
                    )
                    stg = sst.tile([128, QC], F32, tag="st")
                    nc.vector.tensor_copy(stg[:], op[:])
                    nc.sync.dma_start(
                        outT[of * 128:(of + 1) * 128, boff + qs: boff + qs + QC],
                        stg[:],
                    )

            # ---- software pipeline over (batch, chunk) with LAG=1 ----
            seq = [(b, c) for b in range(B) for c in range(NQC)]
            frontend(*seq[0])
            frontend(*seq[1])
            for i, (b, qc) in enumerate(seq):
                attention_chunk(b, qc)
                if i + 2 < len(seq):
                    frontend(*seq[i + 2])
                if qc == NQC - 1:
                    state.pop(b)
    nc.finalize()
    return nc


def _program():
    global _PROGRAM
    if _PROGRAM is None:
        _PROGRAM = _build_program()
    return _PROGRAM


def _rope_tables():
    inv_freq = 1.0 / (ROPE_BASE ** (np.arange(0, HEAD_DIM, 2, dtype=np.float32) / HEAD_DIM))
    t = np.arange(T, dtype=np.float32)
    freqs = np.outer(t, inv_freq).astype(np.float32)        # [T, 32]
    emb = np.concatenate([freqs, freqs], axis=1)            # [T, 64]
    cos = np.cos(emb).astype(np.float32)                    # [T, 64]
    sin = np.sin(emb).astype(np.float32)
    ssin = sin.copy()
    ssin[:, :32] *= -1.0                                    # signed for rotate_half
    cosT = np.ascontiguousarray(cos.T)                      # [64, T]
    ssinT = np.ascontiguousarray(ssin.T)
    cos2 = np.concatenate([cosT] * HPC, axis=0)             # [128, T]
    ssin2 = np.concatenate([ssinT] * HPC, axis=0)
    return cos2, ssin2


def _prep_in_maps(x, W_attn, W_proj):
    x = np.asarray(x, dtype=np.float32)
    W_attn = np.asarray(W_attn, dtype=np.float32)
    W_proj = np.asarray(W_proj, dtype=np.float32)
    xT = np.ascontiguousarray(x.reshape(NT, C).T)
    cos2, ssin2 = _rope_tables()
    in_maps = []
    for i in range(N_CORES):
        cs = i * HF
        in_maps.append({
            "xT": xT,
            "wq": np.ascontiguousarray(W_attn[:, cs:cs + HF]),
            "wk": np.ascontiguousarray(W_attn[:, C + cs:C + cs + HF]),
            "wv": np.ascontiguousarray(W_attn[:, 2 * C + cs:2 * C + cs + HF]),
            "wp": np.ascontiguousarray(W_proj[cs:cs + HF, :]),
            "identd": np.eye(128, dtype=np.float32),
            "onesdd": np.ones((128, 64), dtype=np.float32),
            "cosd": cos2,
            "ssind": ssin2,
        })
    return in_maps


def _run(in_maps, trace=False, **kwargs):
    return run_bass_kernel_spmd(
        _program(), in_maps, core_ids=list(range(N_CORES)), trace=trace, **kwargs
    )


def kernel(x, W_attn, W_proj):
    in_maps = _prep_in_maps(x, W_attn, W_proj)
    res = _run(in_maps)
    acc = np.zeros((C, NT), dtype=np.float32)
    for r in res.results:
        acc += r["outT"]
    return np.ascontiguousarray(acc.T).reshape(B, T, C)



# revision 8
# speedup vs baseline: 1.1797x; 1.1797x over previous
"""Causal self-attention (B=4, T=2048, C=1024, 16 heads, rope) on 8 trn2
NeuronCores, tensor-parallel over heads (2 heads/core).

v2 design vs the v0 baseline:
- bf16 everywhere downstream of the QKV matmul (roped q/k, P, V, ynorm,
  W_proj); QKV + scores accumulate in fp32 PSUM, output stored fp32.
- rope without shuffle-DMAs: tb' = q * ssin_permuted elementwise, then 4
  narrow partition-shifted DVE adds implement rotate_half.
- scalar engine runs ONLY the softmax exp (fp32 PSUM -> bf16 SBUF).
- softmax 1/d via vector.reciprocal + ones-stationary broadcast matmul;
  normalize multiplies write ynorm directly (partition-shifted for h1).
- proj PSUM->SBUF staging on gpsimd; x-in/out-DMAs are 4 large
  host-chunked contiguous transfers per 512-token chunk.
- frontend work is interleaved into the attention ki-loop as closures so
  the PE instruction stream stays dense.
"""

from collections import deque

import ml_dtypes
import numpy as np

import concourse.bacc as bacc
import concourse.mybir as mybir
import concourse.tile as tile
from concourse.bass_utils import run_bass_kernel_spmd

F32 = mybir.dt.float32
F32R = mybir.dt.float32r
BF16 = mybir.dt.bfloat16
AF = mybir.ActivationFunctionType

B, T, C = 4, 2048, 1024
N_HEAD, HEAD_DIM = 16, 64
N_CORES = 8
HPC = N_HEAD // N_CORES          # heads per core = 2
HF = HPC * HEAD_DIM              # per-core head features = 128
NT = B * T                       # 8192 tokens
KT = C // 128                    # 8 contraction tiles for qkv proj
QC = 512                         # query-chunk width
NQC = T // QC                    # 4 query chunks per batch
NCHUNK = B * NQC                 # 16 chunks total
ROPE_BASE = 10000.0
SCALE = 1.0 / 8.0                # 1/sqrt(HEAD_DIM)

_PROGRAM = None


def _patch_act_tables():
    import concourse.bacc as _bacc_mod
    from concourse import hw_specs as _hw

    if getattr(_bacc_mod, "_act_tables_patched", False):
        return
    _orig = _hw.get_activation_tables

    def _patched(arch):
        tabs = {k: set(v) for k, v in _orig(arch).items()}
        if "natural_log_exp_and_others" in tabs:
            for name, fns in tabs.items():
                if name != "natural_log_exp_and_others":
                    fns.discard(AF.Exp)
                    fns.discard(AF.Ln)
        return tabs

    _bacc_mod.get_activation_tables = _patched
    _bacc_mod._act_tables_patched = True


def _build_program():
    _patch_act_tables()
    nc = bacc.Bacc(None, target_bir_lowering=False)

    xd = nc.dram_tensor("xd", [NCHUNK * 128, KT * QC], F32R, kind="ExternalInput")
    wq = nc.dram_tensor("wq", [C, HF], F32R, kind="ExternalInput")
    wk = nc.dram_tensor("wk", [C, HF], F32R, kind="ExternalInput")
    wv = nc.dram_tensor("wv", [C, HF], F32R, kind="ExternalInput")
    wpd = nc.dram_tensor("wpd", [HF, C], BF16, kind="ExternalInput")
    identd = nc.dram_tensor("identd", [128, 128], BF16, kind="ExternalInput")
    onesd = nc.dram_tensor("onesd", [128, 64], F32R, kind="ExternalInput")
    onesbfd = nc.dram_tensor("onesbfd", [128, 32], BF16, kind="ExternalInput")
    cosd = nc.dram_tensor("cosd", [HF, T], F32, kind="ExternalInput")
    ssinpd = nc.dram_tensor("ssinpd", [HF, T], F32, kind="ExternalInput")
    od = nc.dram_tensor("od", [NCHUNK * 128, KT * QC], F32, kind="ExternalOutput")

    with tile.TileContext(nc) as tc:
        with (
            tc.tile_pool(name="const", bufs=1) as cpool,
            tc.tile_pool(name="sx", bufs=2) as sx,
            tc.tile_pool(name="srope", bufs=2) as srope,
            tc.tile_pool(name="stmp", bufs=2) as stmp,
            tc.tile_pool(name="svt", bufs=2) as svt,
            tc.tile_pool(name="sv", bufs=2) as sv,
            tc.tile_pool(name="spt", bufs=3) as spt,
            tc.tile_pool(name="snrm", bufs=2) as snrm,
            tc.tile_pool(name="sy", bufs=2) as sy,
            tc.tile_pool(name="sst", bufs=3) as sst,
            tc.tile_pool(name="pmm", bufs=2, space="PSUM") as pmm,
            tc.tile_pool(name="psc", bufs=2, space="PSUM") as psc,
            tc.tile_pool(name="py", bufs=2, space="PSUM") as py,
        ):
            # ---- constants (DMA order = first-use order) ----
            wqs = cpool.tile([128, C], F32R, tag="wqs")
            wks = cpool.tile([128, C], F32R, tag="wks")
            wvs = cpool.tile([128, C], F32R, tag="wvs")
            for kt in range(KT):
                nc.gpsimd.dma_start(wqs[:, kt * HF:(kt + 1) * HF], wq[kt * 128:(kt + 1) * 128, :])
                nc.gpsimd.dma_start(wks[:, kt * HF:(kt + 1) * HF], wk[kt * 128:(kt + 1) * 128, :])
                nc.gpsimd.dma_start(wvs[:, kt * HF:(kt + 1) * HF], wv[kt * 128:(kt + 1) * 128, :])
            cost = cpool.tile([128, T], F32, tag="cost")
            nc.gpsimd.dma_start(cost[:], cosd[:])
            ssinpt = cpool.tile([128, T], F32, tag="ssinpt")
            nc.gpsimd.dma_start(ssinpt[:], ssinpd[:])
            ident = cpool.tile([128, 128], BF16, tag="ident")
            nc.gpsimd.dma_start(ident[:], identd[:])
            onest = cpool.tile([128, 64], F32R, tag="onest")
            nc.gpsimd.dma_start(onest[:], onesd[:])
            onesbf = cpool.tile([128, 32], BF16, tag="onesbf")
            nc.gpsimd.dma_start(onesbf[:], onesbfd[:])
            wps = cpool.tile([128, C], BF16, tag="wps")
            nc.gpsimd.dma_start(wps[:], wpd[:])

            state = {}

            def fe_closures(ci):
                """Frontend for chunk ci as a list of closures (interleaved
                into the attention ki loop to keep the PE stream dense)."""
                b, c = divmod(ci, NQC)
                ops = []

                def c_x():
                    if c == 0:
                        state[b] = {
                            "rope_q": srope.tile([128, T], BF16, tag="rope_q", name=f"ropeq{b}"),
                            "rope_k": srope.tile([128, T], BF16, tag="rope_k", name=f"ropek{b}"),
                            "vv": sv.tile([128, (T // 128) * 130], BF16, tag="vv", name=f"vv{b}"),
                        }
                        nc.sync.dma_start(
                            state[b]["vv"][:, 64:(T // 128) * 130:65], onesbf[:, 0:32])
                    st = state[b]
                    st["xs"] = sx.tile([128, KT * QC], F32R, tag="xs", name=f"xs{ci}")
                    for j in range(4):
                        nc.sync.dma_start(
                            st["xs"][:, j * 1024:(j + 1) * 1024],
                            xd[ci * 128:(ci + 1) * 128, j * 1024:(j + 1) * 1024],
                        )
                ops.append(c_x)

                cc = slice(c * QC, (c + 1) * QC)

                def mk_kind(wslab, kind):
                    def c_mm():
                        st = state[b]
                        ps = pmm.tile([128, QC], F32, tag="mm", name=f"qkv{ci}_{kind}")
                        st[f"ps_{kind}"] = ps
                        for kt in range(KT):
                            nc.tensor.matmul(
                                ps[:], wslab[:, kt * HF:(kt + 1) * HF],
                                st["xs"][:, kt * QC:(kt + 1) * QC],
                                start=(kt == 0), stop=(kt == KT - 1),
                            )
                    return c_mm

                def mk_rope(kind):
                    def c_rope():
                        st = state[b]
                        ps = st[f"ps_{kind}"]
                        dst = st["rope_q"] if kind == "q" else st["rope_k"]
                        ta = stmp.tile([128, QC], BF16, tag="ta", name=f"ta{ci}{kind}")
                        tbp = stmp.tile([128, QC], BF16, tag="tb", name=f"tb{ci}{kind}")
                        tb = stmp.tile([128, QC], BF16, tag="tb2", name=f"tc{ci}{kind}")
                        nc.vector.tensor_mul(ta[:], ps[:], cost[:, cc])
                        nc.vector.tensor_mul(tbp[:], ps[:], ssinpt[:, cc])
                        # rotate_half: tb[i] = tbp[swap32(i)], via SBUF->SBUF DMA
                        for h in range(HPC):
                            hb = h * 64
                            nc.sync.dma_start(tb[hb:hb + 32, :], tbp[hb + 32:hb + 64, :])
                            nc.sync.dma_start(tb[hb + 32:hb + 64, :], tbp[hb:hb + 32, :])
                        nc.gpsimd.tensor_add(dst[:, cc], ta[:], tb[:])
                    return c_rope

                ops.append(mk_kind(wqs, "q"))
                ops.append(mk_rope("q"))
                ops.append(mk_kind(wks, "k"))
                ops.append(mk_rope("k"))
                ops.append(mk_kind(wvs, "v"))

                def c_v():
                    st = state[b]
                    ps = st["ps_v"]
                    vtc = svt.tile([128, QC], BF16, tag="vtc", name=f"vt{ci}")
                    nc.vector.tensor_copy(vtc[:], ps[:])
                    pst = pmm.tile([128, QC], BF16, tag="mm", name=f"pst{ci}")
                    for t4 in range(4):
                        nc.tensor.transpose(
                            pst[:, t4 * 128:(t4 + 1) * 128],
                            vtc[:, t4 * 128:(t4 + 1) * 128], ident[:])
                    vv = st["vv"]
                    vvr = vv.rearrange("p (a b) -> p a b", b=130).rearrange(
                        "p a (c d) -> p a c d", d=65)
                    pstr = pst.rearrange("p (a b) -> p a b", b=128).rearrange(
                        "p a (c d) -> p a c d", d=64)
                    for half in range(2):
                        ki0 = 4 * c + 2 * half
                        nc.vector.tensor_copy(
                            vvr[:, ki0:ki0 + 2, 0:2, 0:64],
                            pstr[:, 2 * half:2 * half + 2, :, :],
                        )
                ops.append(c_v)
                return ops

            def attention_chunk(ci, fe_q):
                b, qc = divmod(ci, NQC)
                st = state[b]
                rope_q, rope_k, vv = st["rope_q"], st["rope_k"], st["vv"]
                qs = qc * QC
                nki = (qs + QC) // 128
                y_aug = [py.tile([65, QC], F32, tag="y", name=f"y{ci}_{h}")
                         for h in range(HPC)]
                for ki in range(nki):
                    c0 = max(0, ki * 128 - qs)
                    diag = ki * 128 >= qs
                    sc = psc.tile([128, 2 * QC], F32, tag="sc", name=f"sc{ci}_{ki}")
                    for h in range(HPC):
                        hb = h * 64
                        nc.tensor.matmul(
                            sc[:, h * QC + c0:(h + 1) * QC],
                            rope_k[hb:hb + 64, ki * 128:(ki + 1) * 128],
                            rope_q[hb:hb + 64, qs + c0:qs + QC],
                            start=True, stop=True,
                        )
                    pt = spt.tile([128, 2 * QC], BF16, tag="pt", name=f"pt{ci}_{ki}")
                    nc.scalar.activation(
                        pt[:, c0:2 * QC], sc[:, c0:2 * QC], AF.Exp, scale=SCALE,
                    )
                    for h in range(HPC):
                        if diag:
                            nc.gpsimd.affine_select(
                                pt[:, h * QC + c0: h * QC + c0 + 128],
                                pt[:, h * QC + c0: h * QC + c0 + 128],
                                pattern=[[1, 128]],
                                compare_op=mybir.AluOpType.is_ge,
                                fill=0.0, base=0, channel_multiplier=-1,
                            )
                        nc.tensor.matmul(
                            y_aug[h][0:65, c0:QC],
                            vv[:, ki * 130 + 65 * h: ki * 130 + 65 * h + 65],
                            pt[:, h * QC + c0: (h + 1) * QC],
                            start=(ki == 0),
                            stop=(ki == nki - 1),
                        )
                    if fe_q:
                        fe_q.popleft()()
                while fe_q:
                    fe_q.popleft()()
                # ---- normalize: ynorm = y / d, bf16 ----
                rec = snrm.tile([128, 2 * QC], F32R, tag="rec", name=f"rec{ci}")
                ynorm = sy.tile([128, QC], BF16, tag="ynorm", name=f"yn{ci}")
                bcs = snrm.tile([128, 2 * QC], F32R, tag="bcs", name=f"bcs{ci}")
                for h in range(HPC):
                    with nc.allow_low_precision(reason="f32r == f32 bitwise"):
                        nc.vector.reciprocal(
                            rec[64:65, h * QC:(h + 1) * QC], y_aug[h][64:65, :])
                    bc = pmm.tile([128, QC], F32, tag="mm", name=f"bc{ci}_{h}")
                    nc.tensor.matmul(
                        bc[0:64, :], onest[64:65, 0:64],
                        rec[64:65, h * QC:(h + 1) * QC],
                        start=True, stop=True,
                    )
                    nc.vector.tensor_copy(bcs[0:64, h * QC:(h + 1) * QC], bc[0:64, :])
                    nc.vector.tensor_mul(
                        ynorm[h * 64:(h + 1) * 64, :], y_aug[h][0:64, :],
                        bcs[0:64, h * QC:(h + 1) * QC])
                # ---- output projection ----
                stg = sst.tile([128, KT * QC], F32, tag="st", name=f"stg{ci}")
                for j in range(4):
                    op = psc.tile([128, 2 * QC], F32, tag="sc", name=f"op{ci}_{j}")
                    for s2 in range(2):
                        of = 2 * j + s2
                        nc.tensor.matmul(
                            op[:, s2 * QC:(s2 + 1) * QC],
                            wps[:, of * 128:(of + 1) * 128], ynorm[:],
                            start=True, stop=True,
                        )
                    if j % 2 == 0:
                        nc.vector.tensor_copy(stg[:, j * 1024:(j + 1) * 1024], op[:])
                    else:
                        nc.scalar.copy(stg[:, j * 1024:(j + 1) * 1024], op[:])
                    nc.sync.dma_start(
                        od[ci * 128:(ci + 1) * 128, j * 1024:(j + 1) * 1024],
                        stg[:, j * 1024:(j + 1) * 1024],
                    )
                if qc == NQC - 1:
                    state.pop(b)

            # ---- software pipeline: attention(i) interleaves frontend(i+2) ----
            fe_q = deque()
            for op in fe_closures(0):
                op()
            for op in fe_closures(1):
                op()
            for i in range(NCHUNK):
                if i + 2 < NCHUNK:
                    fe_q.extend(fe_closures(i + 2))
                attention_chunk(i, fe_q)
    nc.finalize()
    return nc


def _program():
    global _PROGRAM
    if _PROGRAM is None:
        _PROGRAM = _build_program()
    return _PROGRAM


def _rope_tables():
    inv_freq = 1.0 / (ROPE_BASE ** (np.arange(0, HEAD_DIM, 2, dtype=np.float32) / HEAD_DIM))
    t = np.arange(T, dtype=np.float32)
    freqs = np.outer(t, inv_freq).astype(np.float32)        # [T, 32]
    emb = np.concatenate([freqs, freqs], axis=1)            # [T, 64]
    cos = np.cos(emb).astype(np.float32)                    # [T, 64]
    sin_fr = np.sin(freqs).astype(np.float32)               # [T, 32]
    # ssin_permuted: [:32] = +sin_fr, [32:] = -sin_fr
    ssinp = np.concatenate([sin_fr, -sin_fr], axis=1)       # [T, 64]
    cosT = np.ascontiguousarray(cos.T)                      # [64, T]
    ssinpT = np.ascontiguousarray(ssinp.T)
    cos2 = np.concatenate([cosT] * HPC, axis=0)             # [128, T]
    ssinp2 = np.concatenate([ssinpT] * HPC, axis=0)
    return cos2, ssinp2


def _prep_in_maps(x, W_attn, W_proj):
    x = np.asarray(x, dtype=np.float32)
    W_attn = np.asarray(W_attn, dtype=np.float32)
    W_proj = np.asarray(W_proj, dtype=np.float32)
    # xd[(b*NQC+qc)*128 + p, kt*QC + j] = x[b, qc*QC+j, kt*128+p]
    xd = np.ascontiguousarray(
        x.reshape(B, NQC, QC, KT, 128).transpose(0, 1, 4, 3, 2)
        .reshape(NCHUNK * 128, KT * QC))
    cos2, ssinp2 = _rope_tables()
    in_maps = []
    for i in range(N_CORES):
        cs = i * HF
        in_maps.append({
            "xd": xd,
            "wq": np.ascontiguousarray(W_attn[:, cs:cs + HF]),
            "wk": np.ascontiguousarray(W_attn[:, C + cs:C + cs + HF]),
            "wv": np.ascontiguousarray(W_attn[:, 2 * C + cs:2 * C + cs + HF]),
            "wpd": np.ascontiguousarray(W_proj[cs:cs + HF, :]).astype(ml_dtypes.bfloat16),
            "identd": np.eye(128, dtype=ml_dtypes.bfloat16),
            "onesd": np.ones((128, 64), dtype=np.float32),
            "onesbfd": np.ones((128, 32), dtype=ml_dtypes.bfloat16),
            "cosd": cos2,
            "ssinpd": ssinp2,
        })
    return in_maps


def _gather(res):
    acc = np.zeros((NCHUNK * 128, KT * QC), dtype=np.float32)
    for r in res.results:
        acc += r["od"]
    # od[(b*NQC+qc)*128 + p, of*QC + j] = out[b, qc*QC+j, of*128+p]
    return np.ascontiguousarray(
        acc.reshape(B, NQC, 128, KT, QC).transpose(0, 1, 4, 3, 2)
        .reshape(B, T, C))


def _run(in_maps, trace=False, **kwargs):
    return run_bass_kernel_spmd(
        _program(), in_maps, core_ids=list(range(N_CORES)), trace=trace, **kwargs
    )


def kernel(x, W_attn, W_proj):
    in_maps = _prep_in_maps(x, W_attn, W_proj)
    res = _run(in_maps)
    return _gather(res)


# revision 15
# speedup vs baseline: 1.3809x; 1.1705x over previous
"""Causal self-attention (B=4, T=2048, C=1024, 16 heads, rope) on 8 trn2
NeuronCores, tensor-parallel over heads (2 heads/core).

v2 design vs the v0 baseline:
- bf16 everywhere downstream of the QKV matmul (roped q/k, P, V, ynorm,
  W_proj); QKV + scores accumulate in fp32 PSUM, output stored fp32.
- rope without shuffle-DMAs: tb' = q * ssin_permuted elementwise, then 4
  narrow partition-shifted DVE adds implement rotate_half.
- scalar engine runs ONLY the softmax exp (fp32 PSUM -> bf16 SBUF).
- softmax 1/d via vector.reciprocal + ones-stationary broadcast matmul;
  normalize multiplies write ynorm directly (partition-shifted for h1).
- proj PSUM->SBUF staging on gpsimd; x-in/out-DMAs are 4 large
  host-chunked contiguous transfers per 512-token chunk.
- frontend work is interleaved into the attention ki-loop as closures so
  the PE instruction stream stays dense.
"""

from collections import deque

import ml_dtypes
import numpy as np

import concourse.bacc as bacc
import concourse.mybir as mybir
import concourse.tile as tile
from concourse.bass_utils import run_bass_kernel_spmd

F32 = mybir.dt.float32
F32R = mybir.dt.float32r
BF16 = mybir.dt.bfloat16
AF = mybir.ActivationFunctionType

B, T, C = 4, 2048, 1024
N_HEAD, HEAD_DIM = 16, 64
N_CORES = 8
HPC = N_HEAD // N_CORES          # heads per core = 2
HF = HPC * HEAD_DIM              # per-core head features = 128
NT = B * T                       # 8192 tokens
KT = C // 128                    # 8 contraction tiles for qkv proj
QC = 512                         # query-chunk width
NQC = T // QC                    # 4 query chunks per batch
NCHUNK = B * NQC                 # 16 chunks total
ROPE_BASE = 10000.0
SCALE = 1.0 / 8.0                # 1/sqrt(HEAD_DIM)

_PROGRAM = None


def _patch_act_tables():
    import concourse.bacc as _bacc_mod
    from concourse import hw_specs as _hw

    if getattr(_bacc_mod, "_act_tables_patched", False):
        return
    _orig = _hw.get_activation_tables

    def _patched(arch):
        tabs = {k: set(v) for k, v in _orig(arch).items()}
        if "natural_log_exp_and_others" in tabs:
            for name, fns in tabs.items():
                if name != "natural_log_exp_and_others":
                    fns.discard(AF.Exp)
                    fns.discard(AF.Ln)
        return tabs

    _bacc_mod.get_activation_tables = _patched
    _bacc_mod._act_tables_patched = True


def _build_program():
    _patch_act_tables()
    nc = bacc.Bacc(None, target_bir_lowering=False)

    xd = nc.dram_tensor("xd", [NCHUNK * 128, KT * QC], F32R, kind="ExternalInput")
    wq = nc.dram_tensor("wq", [C, HF], F32R, kind="ExternalInput")
    wk = nc.dram_tensor("wk", [C, HF], F32R, kind="ExternalInput")
    wv = nc.dram_tensor("wv", [C, HF], F32R, kind="ExternalInput")
    wpd = nc.dram_tensor("wpd", [HF, C], BF16, kind="ExternalInput")
    identd = nc.dram_tensor("identd", [128, 128], BF16, kind="ExternalInput")
    onesd = nc.dram_tensor("onesd", [128, 64], F32R, kind="ExternalInput")
    onesbfd = nc.dram_tensor("onesbfd", [128, 32], BF16, kind="ExternalInput")
    cosd = nc.dram_tensor("cosd", [HF, T], F32, kind="ExternalInput")
    ssinpd = nc.dram_tensor("ssinpd", [HF, T], F32, kind="ExternalInput")
    od = nc.dram_tensor("od", [NCHUNK * 128, KT * QC], F32, kind="ExternalOutput")

    with tile.TileContext(nc) as tc:
        with (
            tc.tile_pool(name="const", bufs=1) as cpool,
            tc.tile_pool(name="sx", bufs=2) as sx,
            tc.tile_pool(name="srope", bufs=2) as srope,
            tc.tile_pool(name="stmp", bufs=2) as stmp,
            tc.tile_pool(name="svt", bufs=2) as svt,
            tc.tile_pool(name="sv", bufs=2) as sv,
            tc.tile_pool(name="spt", bufs=3) as spt,
            tc.tile_pool(name="snrm", bufs=2) as snrm,
            tc.tile_pool(name="sy", bufs=2) as sy,
            tc.tile_pool(name="sst", bufs=3) as sst,
            tc.tile_pool(name="pmm", bufs=2, space="PSUM") as pmm,
            tc.tile_pool(name="psc", bufs=2, space="PSUM") as psc,
            tc.tile_pool(name="py", bufs=2, space="PSUM") as py,
        ):
            # ---- constants (DMA order = first-use order) ----
            wqs = cpool.tile([128, C], F32R, tag="wqs")
            wks = cpool.tile([128, C], F32R, tag="wks")
            wvs = cpool.tile([128, C], F32R, tag="wvs")
            for kt in range(KT):
                nc.gpsimd.dma_start(wqs[:, kt * HF:(kt + 1) * HF], wq[kt * 128:(kt + 1) * 128, :])
                nc.gpsimd.dma_start(wks[:, kt * HF:(kt + 1) * HF], wk[kt * 128:(kt + 1) * 128, :])
                nc.gpsimd.dma_start(wvs[:, kt * HF:(kt + 1) * HF], wv[kt * 128:(kt + 1) * 128, :])
            cost = cpool.tile([128, T], F32, tag="cost")
            nc.gpsimd.dma_start(cost[:], cosd[:])
            ssinpt = cpool.tile([128, T], F32, tag="ssinpt")
            nc.gpsimd.dma_start(ssinpt[:], ssinpd[:])
            ident = cpool.tile([128, 128], BF16, tag="ident")
            nc.gpsimd.dma_start(ident[:], identd[:])
            onest = cpool.tile([128, 64], F32R, tag="onest")
            nc.gpsimd.dma_start(onest[:], onesd[:])
            onesbf = cpool.tile([128, 32], BF16, tag="onesbf")
            nc.gpsimd.dma_start(onesbf[:], onesbfd[:])
            wps = cpool.tile([128, C], BF16, tag="wps")
            nc.gpsimd.dma_start(wps[:], wpd[:])

            state = {}

            def fe_closures(ci):
                """Frontend for chunk ci as a list of closures (interleaved
                into the attention ki loop to keep the PE stream dense)."""
                b, c = divmod(ci, NQC)
                ops = []

                def c_x():
                    if c == 0:
                        state[b] = {
                            "rope_q": srope.tile([128, T], BF16, tag="rope_q", name=f"ropeq{b}"),
                            "rope_k": srope.tile([128, T], BF16, tag="rope_k", name=f"ropek{b}"),
                            "vv": sv.tile([128, (T // 128) * 130], BF16, tag="vv", name=f"vv{b}"),
                        }
                        nc.sync.dma_start(
                            state[b]["vv"][:, 64:(T // 128) * 130:65], onesbf[:, 0:32])
                    st = state[b]
                    st["xs"] = sx.tile([128, KT * QC], F32R, tag="xs", name=f"xs{ci}")
                    for j in range(4):
                        nc.sync.dma_start(
                            st["xs"][:, j * 1024:(j + 1) * 1024],
                            xd[ci * 128:(ci + 1) * 128, j * 1024:(j + 1) * 1024],
                        )
                ops.append(c_x)

                cc = slice(c * QC, (c + 1) * QC)

                def mk_kind(wslab, kind):
                    def c_mm():
                        st = state[b]
                        ps = pmm.tile([128, QC], F32, tag="mm", name=f"qkv{ci}_{kind}")
                        st[f"ps_{kind}"] = ps
                        for kt in range(KT):
                            nc.tensor.matmul(
                                ps[:], wslab[:, kt * HF:(kt + 1) * HF],
                                st["xs"][:, kt * QC:(kt + 1) * QC],
                                start=(kt == 0), stop=(kt == KT - 1),
                            )
                    return c_mm

                def mk_rope(kind):
                    def c_rope():
                        st = state[b]
                        ps = st[f"ps_{kind}"]
                        dst = st["rope_q"] if kind == "q" else st["rope_k"]
                        ta = stmp.tile([128, QC], BF16, tag="ta", name=f"ta{ci}{kind}")
                        tbp = stmp.tile([128, QC], BF16, tag="tb", name=f"tb{ci}{kind}")
                        tb = stmp.tile([128, QC], BF16, tag="tb2", name=f"tc{ci}{kind}")
                        nc.vector.tensor_mul(ta[:], ps[:], cost[:, cc])
                        nc.vector.tensor_mul(tbp[:], ps[:], ssinpt[:, cc])
                        # rotate_half: tb[i] = tbp[swap32(i)], partition-shifted copies
                        for h in range(HPC):
                            hb = h * 64
                            nc.vector.tensor_copy(tb[hb:hb + 32, :], tbp[hb + 32:hb + 64, :])
                            nc.vector.tensor_copy(tb[hb + 32:hb + 64, :], tbp[hb:hb + 32, :])
                        nc.gpsimd.tensor_add(dst[:, cc], ta[:], tb[:])
                    return c_rope

                ops.append(mk_kind(wqs, "q"))
                ops.append(mk_rope("q"))
                ops.append(mk_kind(wks, "k"))
                ops.append(mk_rope("k"))
                ops.append(mk_kind(wvs, "v"))

                def c_v():
                    st = state[b]
                    ps = st["ps_v"]
                    vtc = svt.tile([128, QC], BF16, tag="vtc", name=f"vt{ci}")
                    nc.vector.tensor_copy(vtc[:], ps[:])
                    pst = pmm.tile([128, QC], BF16, tag="mm", name=f"pst{ci}")
                    for t4 in range(4):
                        nc.tensor.transpose(
                            pst[:, t4 * 128:(t4 + 1) * 128],
                            vtc[:, t4 * 128:(t4 + 1) * 128], ident[:])
                    vv = st["vv"]
                    vvr = vv.rearrange("p (a b) -> p a b", b=130).rearrange(
                        "p a (c d) -> p a c d", d=65)
                    pstr = pst.rearrange("p (a b) -> p a b", b=128).rearrange(
                        "p a (c d) -> p a c d", d=64)
                    for half in range(2):
                        ki0 = 4 * c + 2 * half
                        nc.vector.tensor_copy(
                            vvr[:, ki0:ki0 + 2, 0:2, 0:64],
                            pstr[:, 2 * half:2 * half + 2, :, :],
                        )
                ops.append(c_v)
                return ops

            def attention_chunk(ci, fe_q):
                b, qc = divmod(ci, NQC)
                st = state[b]
                rope_q, rope_k, vv = st["rope_q"], st["rope_k"], st["vv"]
                qs = qc * QC
                nki = (qs + QC) // 128
                y_aug = [py.tile([65, QC], F32, tag="y", name=f"y{ci}_{h}")
                         for h in range(HPC)]
                for ki in range(nki):
                    c0 = max(0, ki * 128 - qs)
                    diag = ki * 128 >= qs
                    sc = psc.tile([128, 2 * QC], F32, tag="sc", name=f"sc{ci}_{ki}")
                    for h in range(HPC):
                        hb = h * 64
                        nc.tensor.matmul(
                            sc[:, h * QC + c0:(h + 1) * QC],
                            rope_k[hb:hb + 64, ki * 128:(ki + 1) * 128],
                            rope_q[hb:hb + 64, qs + c0:qs + QC],
                            start=True, stop=True,
                        )
                    pt = spt.tile([128, 2 * QC], BF16, tag="pt", name=f"pt{ci}_{ki}")
                    if c0 == 0:
                        nc.scalar.activation(
                            pt[:, 0:2 * QC], sc[:, 0:2 * QC], AF.Exp, scale=SCALE,
                        )
                    else:
                        for h in range(HPC):
                            nc.scalar.activation(
                                pt[:, h * QC + c0:(h + 1) * QC],
                                sc[:, h * QC + c0:(h + 1) * QC], AF.Exp, scale=SCALE,
                            )
                    for h in range(HPC):
                        if diag:
                            nc.gpsimd.affine_select(
                                pt[:, h * QC + c0: h * QC + c0 + 128],
                                pt[:, h * QC + c0: h * QC + c0 + 128],
                                pattern=[[1, 128]],
                                compare_op=mybir.AluOpType.is_ge,
                                fill=0.0, base=0, channel_multiplier=-1,
                            )
                        nc.tensor.matmul(
                            y_aug[h][0:65, c0:QC],
                            vv[:, ki * 130 + 65 * h: ki * 130 + 65 * h + 65],
                            pt[:, h * QC + c0: (h + 1) * QC],
                            start=(ki == 0),
                            stop=(ki == nki - 1),
                        )
                    if fe_q:
                        fe_q.popleft()()
                while fe_q:
                    fe_q.popleft()()
                # ---- normalize: ynorm = y / d, bf16 ----
                rec = snrm.tile([128, 2 * QC], F32R, tag="rec", name=f"rec{ci}")
                lnt = snrm.tile([128, 2 * QC], F32R, tag="lnt", name=f"lnt{ci}")
                ynorm = sy.tile([128, QC], BF16, tag="ynorm", name=f"yn{ci}")
                bcs = snrm.tile([128, 2 * QC], F32R, tag="bcs", name=f"bcs{ci}")
                for h in range(HPC):
                    nc.scalar.activation(
                        lnt[64:65, h * QC:(h + 1) * QC], y_aug[h][64:65, :], AF.Ln)
                    nc.scalar.activation(
                        rec[64:65, h * QC:(h + 1) * QC],
                        lnt[64:65, h * QC:(h + 1) * QC], AF.Exp, scale=-1.0)
                    bc = pmm.tile([128, QC], F32, tag="mm", name=f"bc{ci}_{h}")
                    nc.tensor.matmul(
                        bc[0:64, :], onest[64:65, 0:64],
                        rec[64:65, h * QC:(h + 1) * QC],
                        start=True, stop=True,
                    )
                    nc.vector.tensor_copy(bcs[0:64, h * QC:(h + 1) * QC], bc[0:64, :])
                    nc.vector.tensor_mul(
                        ynorm[h * 64:(h + 1) * 64, :], y_aug[h][0:64, :],
                        bcs[0:64, h * QC:(h + 1) * QC])
                # ---- output projection ----
                stg = sst.tile([128, KT * QC], F32, tag="st", name=f"stg{ci}")
                for j in range(4):
                    op = psc.tile([128, 2 * QC], F32, tag="sc", name=f"op{ci}_{j}")
                    for s2 in range(2):
                        of = 2 * j + s2
                        nc.tensor.matmul(
                            op[:, s2 * QC:(s2 + 1) * QC],
                            wps[:, of * 128:(of + 1) * 128], ynorm[:],
                            start=True, stop=True,
                        )
                    if j % 2 == 0:
                        nc.vector.tensor_copy(stg[:, j * 1024:(j + 1) * 1024], op[:])
                    else:
                        nc.scalar.copy(stg[:, j * 1024:(j + 1) * 1024], op[:])
                    nc.sync.dma_start(
                        od[ci * 128:(ci + 1) * 128, j * 1024:(j + 1) * 1024],
                        stg[:, j * 1024:(j + 1) * 1024],
                    )
                if qc == NQC - 1:
                    state.pop(b)

            # ---- software pipeline: attention(i) interleaves frontend(i+2) ----
            fe_q = deque()
            for op in fe_closures(0):
                op()
            for op in fe_closures(1):
                op()
            for i in range(NCHUNK):
                if i + 2 < NCHUNK:
                    fe_q.extend(fe_closures(i + 2))
                attention_chunk(i, fe_q)
    nc.finalize()
    return nc


def _program():
    global _PROGRAM
    if _PROGRAM is None:
        _PROGRAM = _build_program()
    return _PROGRAM


def _rope_tables():
    inv_freq = 1.0 / (ROPE_BASE ** (np.arange(0, HEAD_DIM, 2, dtype=np.float32) / HEAD_DIM))
    t = np.arange(T, dtype=np.float32)
    freqs = np.outer(t, inv_freq).astype(np.float32)        # [T, 32]
    emb = np.concatenate([freqs, freqs], axis=1)            # [T, 64]
    cos = np.cos(emb).astype(np.float32)                    # [T, 64]
    sin_fr = np.sin(freqs).astype(np.float32)               # [T, 32]
    # ssin_permuted: [:32] = +sin_fr, [32:] = -sin_fr
    ssinp = np.concatenate([sin_fr, -sin_fr], axis=1)       # [T, 64]
    cosT = np.ascontiguousarray(cos.T)                      # [64, T]
    ssinpT = np.ascontiguousarray(ssinp.T)
    cos2 = np.concatenate([cosT] * HPC, axis=0)             # [128, T]
    ssinp2 = np.concatenate([ssinpT] * HPC, axis=0)
    return cos2, ssinp2


def _prep_in_maps(x, W_attn, W_proj):
    x = np.asarray(x, dtype=np.float32)
    W_attn = np.asarray(W_attn, dtype=np.float32)
    W_proj = np.asarray(W_proj, dtype=np.float32)
    # xd[(b*NQC+qc)*128 + p, kt*QC + j] = x[b, qc*QC+j, kt*128+p]
    xd = np.ascontiguousarray(
        x.reshape(B, NQC, QC, KT, 128).transpose(0, 1, 4, 3, 2)
        .reshape(NCHUNK * 128, KT * QC))
    cos2, ssinp2 = _rope_tables()
    in_maps = []
    for i in range(N_CORES):
        cs = i * HF
        in_maps.append({
            "xd": xd,
            "wq": np.ascontiguousarray(W_attn[:, cs:cs + HF]),
            "wk": np.ascontiguousarray(W_attn[:, C + cs:C + cs + HF]),
            "wv": np.ascontiguousarray(W_attn[:, 2 * C + cs:2 * C + cs + HF]),
            "wpd": np.ascontiguousarray(W_proj[cs:cs + HF, :]).astype(ml_dtypes.bfloat16),
            "identd": np.eye(128, dtype=ml_dtypes.bfloat16),
            "onesd": np.ones((128, 64), dtype=np.float32),
            "onesbfd": np.ones((128, 32), dtype=ml_dtypes.bfloat16),
            "cosd": cos2,
            "ssinpd": ssinp2,
        })
    return in_maps


def _gather(res):
    acc = np.zeros((NCHUNK * 128, KT * QC), dtype=np.float32)
    for r in res.results:
        acc += r["od"]
    # od[(b*NQC+qc)*128 + p, of*QC + j] = out[b, qc*QC+j, of*128+p]
    return np.ascontiguousarray(
        acc.reshape(B, NQC, 128, KT, QC).transpose(0, 1, 4, 3, 2)
        .reshape(B, T, C))


def _run(in_maps, trace=False, **kwargs):
    return run_bass_kernel_spmd(
        _program(), in_maps, core_ids=list(range(N_CORES)), trace=trace, **kwargs
    )


def kernel(x, W_attn, W_proj):
    in_maps = _prep_in_maps(x, W_attn, W_proj)
    res = _run(in_maps)
    return _gather(res)


# revision 22
# speedup vs baseline: 1.3852x; 1.0032x over previous
"""Causal self-attention (B=4, T=2048, C=1024, 16 heads, rope) on 8 trn2
NeuronCores, tensor-parallel over heads (2 heads/core).

v2 design vs the v0 baseline:
- bf16 everywhere downstream of the QKV matmul (roped q/k, P, V, ynorm,
  W_proj); QKV + scores accumulate in fp32 PSUM, output stored fp32.
- rope without shuffle-DMAs: tb' = q * ssin_permuted elementwise, then 4
  narrow partition-shifted DVE adds implement rotate_half.
- scalar engine runs ONLY the softmax exp (fp32 PSUM -> bf16 SBUF).
- softmax 1/d via vector.reciprocal + ones-stationary broadcast matmul;
  normalize multiplies write ynorm directly (partition-shifted for h1).
- proj PSUM->SBUF staging on gpsimd; x-in/out-DMAs are 4 large
  host-chunked contiguous transfers per 512-token chunk.
- frontend work is interleaved into the attention ki-loop as closures so
  the PE instruction stream stays dense.
"""

from collections import deque

import ml_dtypes
import numpy as np

import concourse.bacc as bacc
import concourse.mybir as mybir
import concourse.tile as tile
from concourse.bass_utils import run_bass_kernel_spmd

F32 = mybir.dt.float32
F32R = mybir.dt.float32r
BF16 = mybir.dt.bfloat16
AF = mybir.ActivationFunctionType

B, T, C = 4, 2048, 1024
N_HEAD, HEAD_DIM = 16, 64
N_CORES = 8
HPC = N_HEAD // N_CORES          # heads per core = 2
HF = HPC * HEAD_DIM              # per-core head features = 128
NT = B * T                       # 8192 tokens
KT = C // 128                    # 8 contraction tiles for qkv proj
QC = 512                         # query-chunk width
NQC = T // QC                    # 4 query chunks per batch
NCHUNK = B * NQC                 # 16 chunks total
ROPE_BASE = 10000.0
SCALE = 1.0 / 8.0                # 1/sqrt(HEAD_DIM)

_PROGRAM = None


def _patch_act_tables():
    import concourse.bacc as _bacc_mod
    from concourse import hw_specs as _hw

    if getattr(_bacc_mod, "_act_tables_patched", False):
        return
    _orig = _hw.get_activation_tables

    def _patched(arch):
        tabs = {k: set(v) for k, v in _orig(arch).items()}
        if "natural_log_exp_and_others" in tabs:
            for name, fns in tabs.items():
                if name != "natural_log_exp_and_others":
                    fns.discard(AF.Exp)
                    fns.discard(AF.Ln)
        return tabs

    _bacc_mod.get_activation_tables = _patched
    _bacc_mod._act_tables_patched = True


def _build_program():
    _patch_act_tables()
    nc = bacc.Bacc(None, target_bir_lowering=False)

    xd = nc.dram_tensor("xd", [NCHUNK * 128, KT * QC], F32R, kind="ExternalInput")
    wq = nc.dram_tensor("wq", [C, HF], F32R, kind="ExternalInput")
    wk = nc.dram_tensor("wk", [C, HF], F32R, kind="ExternalInput")
    wv = nc.dram_tensor("wv", [C, HF], F32R, kind="ExternalInput")
    wpd = nc.dram_tensor("wpd", [HF, C], BF16, kind="ExternalInput")
    identd = nc.dram_tensor("identd", [128, 128], BF16, kind="ExternalInput")
    onesbfd = nc.dram_tensor("onesbfd", [128, 64], BF16, kind="ExternalInput")
    cosd = nc.dram_tensor("cosd", [HF, T], F32, kind="ExternalInput")
    ssinpd = nc.dram_tensor("ssinpd", [HF, T], F32, kind="ExternalInput")
    od = nc.dram_tensor("od", [NCHUNK * 128, KT * QC], F32, kind="ExternalOutput")

    with tile.TileContext(nc) as tc:
        with (
            tc.tile_pool(name="const", bufs=1) as cpool,
            tc.tile_pool(name="sx", bufs=2) as sx,
            tc.tile_pool(name="srope", bufs=2) as srope,
            tc.tile_pool(name="stmp", bufs=2) as stmp,
            tc.tile_pool(name="svt", bufs=2) as svt,
            tc.tile_pool(name="sv", bufs=2) as sv,
            tc.tile_pool(name="spt", bufs=3) as spt,
            tc.tile_pool(name="snrm", bufs=2) as snrm,
            tc.tile_pool(name="sy", bufs=2) as sy,
            tc.tile_pool(name="sst", bufs=3) as sst,
            tc.tile_pool(name="pmm", bufs=2, space="PSUM") as pmm,
            tc.tile_pool(name="psc", bufs=2, space="PSUM") as psc,
            tc.tile_pool(name="py", bufs=2, space="PSUM") as py,
        ):
            # ---- constants (DMA order = first-use order) ----
            wqs = cpool.tile([128, C], F32R, tag="wqs")
            wks = cpool.tile([128, C], F32R, tag="wks")
            wvs = cpool.tile([128, C], F32R, tag="wvs")
            for kt in range(KT):
                nc.gpsimd.dma_start(wqs[:, kt * HF:(kt + 1) * HF], wq[kt * 128:(kt + 1) * 128, :])
                nc.gpsimd.dma_start(wks[:, kt * HF:(kt + 1) * HF], wk[kt * 128:(kt + 1) * 128, :])
                nc.gpsimd.dma_start(wvs[:, kt * HF:(kt + 1) * HF], wv[kt * 128:(kt + 1) * 128, :])
            cost = cpool.tile([128, T], F32, tag="cost")
            nc.gpsimd.dma_start(cost[:], cosd[:])
            ssinpt = cpool.tile([128, T], F32, tag="ssinpt")
            nc.gpsimd.dma_start(ssinpt[:], ssinpd[:])
            ident = cpool.tile([128, 128], BF16, tag="ident")
            nc.gpsimd.dma_start(ident[:], identd[:])
            onesbf = cpool.tile([128, 64], BF16, tag="onesbf")
            nc.gpsimd.dma_start(onesbf[:], onesbfd[:])
            wps = cpool.tile([128, C], BF16, tag="wps")
            nc.gpsimd.dma_start(wps[:], wpd[:])

            state = {}

            def fe_closures(ci):
                """Frontend for chunk ci as a list of closures (interleaved
                into the attention ki loop to keep the PE stream dense)."""
                b, c = divmod(ci, NQC)
                ops = []

                def c_x():
                    if c == 0:
                        state[b] = {
                            "rope_q": srope.tile([128, T], BF16, tag="rope_q", name=f"ropeq{b}"),
                            "rope_k": srope.tile([128, T], BF16, tag="rope_k", name=f"ropek{b}"),
                            "vv": sv.tile([128, (T // 128) * 130], BF16, tag="vv", name=f"vv{b}"),
                        }
                        nc.sync.dma_start(
                            state[b]["vv"][:, 64:(T // 128) * 130:65], onesbf[:, 0:32])
                    st = state[b]
                    st["xs"] = sx.tile([128, KT * QC], F32R, tag="xs", name=f"xs{ci}")
                    for j in range(4):
                        nc.sync.dma_start(
                            st["xs"][:, j * 1024:(j + 1) * 1024],
                            xd[ci * 128:(ci + 1) * 128, j * 1024:(j + 1) * 1024],
                        )
                ops.append(c_x)

                cc = slice(c * QC, (c + 1) * QC)

                def mk_kind(wslab, kind):
                    def c_mm():
                        st = state[b]
                        ps = pmm.tile([128, QC], F32, tag="mm", name=f"qkv{ci}_{kind}")
                        st[f"ps_{kind}"] = ps
                        for kt in range(KT):
                            nc.tensor.matmul(
                                ps[:], wslab[:, kt * HF:(kt + 1) * HF],
                                st["xs"][:, kt * QC:(kt + 1) * QC],
                                start=(kt == 0), stop=(kt == KT - 1),
                            )
                    return c_mm

                def mk_rope(kind):
                    def c_rope():
                        st = state[b]
                        ps = st[f"ps_{kind}"]
                        dst = st["rope_q"] if kind == "q" else st["rope_k"]
                        ta = stmp.tile([128, QC], BF16, tag="ta", name=f"ta{ci}{kind}")
                        tbp = stmp.tile([128, QC], BF16, tag="tb", name=f"tb{ci}{kind}")
                        tb = stmp.tile([128, QC], BF16, tag="tb2", name=f"tc{ci}{kind}")
                        nc.vector.tensor_mul(ta[:], ps[:], cost[:, cc])
                        nc.vector.tensor_mul(tbp[:], ps[:], ssinpt[:, cc])
                        # rotate_half: tb[i] = tbp[swap32(i)], partition-shifted copies
                        for h in range(HPC):
                            hb = h * 64
                            nc.vector.tensor_copy(tb[hb:hb + 32, :], tbp[hb + 32:hb + 64, :])
                            nc.vector.tensor_copy(tb[hb + 32:hb + 64, :], tbp[hb:hb + 32, :])
                        nc.gpsimd.tensor_add(dst[:, cc], ta[:], tb[:])
                    return c_rope

                ops.append(mk_kind(wqs, "q"))
                ops.append(mk_rope("q"))
                ops.append(mk_kind(wks, "k"))
                ops.append(mk_rope("k"))
                ops.append(mk_kind(wvs, "v"))

                def c_v():
                    st = state[b]
                    ps = st["ps_v"]
                    vtc = svt.tile([128, QC], BF16, tag="vtc", name=f"vt{ci}")
                    nc.vector.tensor_copy(vtc[:], ps[:])
                    pst = pmm.tile([128, QC], BF16, tag="mm", name=f"pst{ci}")
                    for t4 in range(4):
                        nc.tensor.transpose(
                            pst[:, t4 * 128:(t4 + 1) * 128],
                            vtc[:, t4 * 128:(t4 + 1) * 128], ident[:])
                    vv = st["vv"]
                    vvr = vv.rearrange("p (a b) -> p a b", b=130).rearrange(
                        "p a (c d) -> p a c d", d=65)
                    pstr = pst.rearrange("p (a b) -> p a b", b=128).rearrange(
                        "p a (c d) -> p a c d", d=64)
                    for half in range(2):
                        ki0 = 4 * c + 2 * half
                        nc.vector.tensor_copy(
                            vvr[:, ki0:ki0 + 2, 0:2, 0:64],
                            pstr[:, 2 * half:2 * half + 2, :, :],
                        )
                ops.append(c_v)
                return ops

            def attention_chunk(ci, fe_q):
                b, qc = divmod(ci, NQC)
                st = state[b]
                rope_q, rope_k, vv = st["rope_q"], st["rope_k"], st["vv"]
                qs = qc * QC
                nki = (qs + QC) // 128
                y_aug = [py.tile([65, QC], F32, tag="y", name=f"y{ci}_{h}")
                         for h in range(HPC)]
                def emit_pv(pt, ki, c0):
                    for h in range(HPC):
                        nc.tensor.matmul(
                            y_aug[h][0:65, c0:QC],
                            vv[:, ki * 130 + 65 * h: ki * 130 + 65 * h + 65],
                            pt[:, h * QC + c0: (h + 1) * QC],
                            start=(ki == 0),
                            stop=(ki == nki - 1),
                        )

                prev_pv = None
                for ki in range(nki):
                    c0 = max(0, ki * 128 - qs)
                    diag = ki * 128 >= qs
                    sc = psc.tile([128, 2 * QC], F32, tag="sc", name=f"sc{ci}_{ki}")
                    for h in range(HPC):
                        hb = h * 64
                        nc.tensor.matmul(
                            sc[:, h * QC + c0:(h + 1) * QC],
                            rope_k[hb:hb + 64, ki * 128:(ki + 1) * 128],
                            rope_q[hb:hb + 64, qs + c0:qs + QC],
                            start=True, stop=True,
                        )
                    pt = spt.tile([128, 2 * QC], BF16, tag="pt", name=f"pt{ci}_{ki}")
                    if c0 == 0:
                        nc.scalar.activation(
                            pt[:, 0:2 * QC], sc[:, 0:2 * QC], AF.Exp, scale=SCALE,
                        )
                    else:
                        for h in range(HPC):
                            nc.scalar.activation(
                                pt[:, h * QC + c0:(h + 1) * QC],
                                sc[:, h * QC + c0:(h + 1) * QC], AF.Exp, scale=SCALE,
                            )
                    if diag:
                        for h in range(HPC):
                            nc.gpsimd.affine_select(
                                pt[:, h * QC + c0: h * QC + c0 + 128],
                                pt[:, h * QC + c0: h * QC + c0 + 128],
                                pattern=[[1, 128]],
                                compare_op=mybir.AluOpType.is_ge,
                                fill=0.0, base=0, channel_multiplier=-1,
                            )
                    # PE fill while exp(ki) runs on scalar
                    if fe_q and len(fe_q) > 2:
                        fe_q.popleft()()
                    if prev_pv is not None:
                        emit_pv(*prev_pv)
                    prev_pv = (pt, ki, c0)
                emit_pv(*prev_pv)
                # ---- normalize: ynorm = y / d, bf16 ----
                rec = snrm.tile([128, 2 * QC], BF16, tag="rec", name=f"rec{ci}")
                lnt = snrm.tile([128, 2 * QC], F32R, tag="lnt", name=f"lnt{ci}")
                ynorm = sy.tile([128, QC], BF16, tag="ynorm", name=f"yn{ci}")
                bcs = snrm.tile([128, 2 * QC], F32R, tag="bcs", name=f"bcs{ci}")
                for h in range(HPC):
                    nc.scalar.activation(
                        lnt[64:65, h * QC:(h + 1) * QC], y_aug[h][64:65, :], AF.Ln)
                    nc.scalar.activation(
                        rec[64:65, h * QC:(h + 1) * QC],
                        lnt[64:65, h * QC:(h + 1) * QC], AF.Exp, scale=-1.0)
                if fe_q:
                    fe_q.popleft()()
                bcp = []
                for h in range(HPC):
                    bc = pmm.tile([128, QC], F32, tag="mm", name=f"bc{ci}_{h}")
                    nc.tensor.matmul(
                        bc[0:64, :], onesbf[64:65, 0:64],
                        rec[64:65, h * QC:(h + 1) * QC],
                        start=True, stop=True,
                    )
                    bcp.append(bc)
                if fe_q:
                    fe_q.popleft()()
                for h in range(HPC):
                    nc.vector.tensor_copy(bcs[0:64, h * QC:(h + 1) * QC], bcp[h][0:64, :])
                    nc.vector.tensor_mul(
                        ynorm[h * 64:(h + 1) * 64, :], y_aug[h][0:64, :],
                        bcs[0:64, h * QC:(h + 1) * QC])
                while fe_q:
                    fe_q.popleft()()
                # ---- output projection ----
                stg = sst.tile([128, KT * QC], F32, tag="st", name=f"stg{ci}")
                for j in range(4):
                    op = psc.tile([128, 2 * QC], F32, tag="sc", name=f"op{ci}_{j}")
                    for s2 in range(2):
                        of = 2 * j + s2
                        nc.tensor.matmul(
                            op[:, s2 * QC:(s2 + 1) * QC],
                            wps[:, of * 128:(of + 1) * 128], ynorm[:],
                            start=True, stop=True,
                        )
                    if j % 2 == 0:
                        nc.vector.tensor_copy(stg[:, j * 1024:(j + 1) * 1024], op[:])
                    else:
                        nc.scalar.copy(stg[:, j * 1024:(j + 1) * 1024], op[:])
                    nc.sync.dma_start(
                        od[ci * 128:(ci + 1) * 128, j * 1024:(j + 1) * 1024],
                        stg[:, j * 1024:(j + 1) * 1024],
                    )
                if qc == NQC - 1:
                    state.pop(b)

            # ---- software pipeline: attention(i) interleaves frontend(i+2) ----
            fe_q = deque()
            for op in fe_closures(0):
                op()
            for op in fe_closures(1):
                op()
            for i in range(NCHUNK):
                if i + 2 < NCHUNK:
                    fe_q.extend(fe_closures(i + 2))
                attention_chunk(i, fe_q)
    nc.finalize()
    return nc


def _program():
    global _PROGRAM
    if _PROGRAM is None:
        _PROGRAM = _build_program()
    return _PROGRAM


def _rope_tables():
    inv_freq = 1.0 / (ROPE_BASE ** (np.arange(0, HEAD_DIM, 2, dtype=np.float32) / HEAD_DIM))
    t = np.arange(T, dtype=np.float32)
    freqs = np.outer(t, inv_freq).astype(np.float32)        # [T, 32]
    emb = np.concatenate([freqs, freqs], axis=1)            # [T, 64]
    cos = np.cos(emb).astype(np.float32)                    # [T, 64]
    sin_fr = np.sin(freqs).astype(np.float32)               # [T, 32]
    # ssin_permuted: [:32] = +sin_fr, [32:] = -sin_fr
    ssinp = np.concatenate([sin_fr, -sin_fr], axis=1)       # [T, 64]
    cosT = np.ascontiguousarray(cos.T)                      # [64, T]
    ssinpT = np.ascontiguousarray(ssinp.T)
    cos2 = np.concatenate([cosT] * HPC, axis=0)             # [128, T]
    ssinp2 = np.concatenate([ssinpT] * HPC, axis=0)
    return cos2, ssinp2


def _prep_in_maps(x, W_attn, W_proj):
    x = np.asarray(x, dtype=np.float32)
    W_attn = np.asarray(W_attn, dtype=np.float32)
    W_proj = np.asarray(W_proj, dtype=np.float32)
    # xd[(b*NQC+qc)*128 + p, kt*QC + j] = x[b, qc*QC+j, kt*128+p]
    xd = np.ascontiguousarray(
        x.reshape(B, NQC, QC, KT, 128).transpose(0, 1, 4, 3, 2)
        .reshape(NCHUNK * 128, KT * QC))
    cos2, ssinp2 = _rope_tables()
    in_maps = []
    for i in range(N_CORES):
        cs = i * HF
        in_maps.append({
            "xd": xd,
            "wq": np.ascontiguousarray(W_attn[:, cs:cs + HF]),
            "wk": np.ascontiguousarray(W_attn[:, C + cs:C + cs + HF]),
            "wv": np.ascontiguousarray(W_attn[:, 2 * C + cs:2 * C + cs + HF]),
            "wpd": np.ascontiguousarray(W_proj[cs:cs + HF, :]).astype(ml_dtypes.bfloat16),
            "identd": np.eye(128, dtype=ml_dtypes.bfloat16),
            "onesbfd": np.ones((128, 64), dtype=ml_dtypes.bfloat16),
            "cosd": cos2,
            "ssinpd": ssinp2,
        })
    return in_maps


def _gather(res):
    acc = np.zeros((NCHUNK * 128, KT * QC), dtype=np.float32)
    for r in res.results:
        acc += r["od"]
    # od[(b*NQC+qc)*128 + p, of*QC + j] = out[b, qc*QC+j, of*128+p]
    return np.ascontiguousarray(
        acc.reshape(B, NQC, 128, KT, QC).transpose(0, 1, 4, 3, 2)
        .reshape(B, T, C))


def _run(in_maps, trace=False, **kwargs):
    return run_bass_kernel_spmd(
        _program(), in_maps, core_ids=list(range(N_CORES)), trace=trace, **kwargs
    )


def kernel(x, W_attn, W_proj):
    in_maps = _prep_in_maps(x, W_attn, W_proj)
    res = _run(in_maps)
    return _gather(res)


# revision 24
# speedup vs baseline: 1.5159x; 1.0943x over previous
"""Causal self-attention (B=4, T=2048, C=1024, 16 heads, rope) on 8 trn2
NeuronCores, tensor-parallel over heads (2 heads/core).

v2 design vs the v0 baseline:
- bf16 everywhere downstream of the QKV matmul (roped q/k, P, V, ynorm,
  W_proj); QKV + scores accumulate in fp32 PSUM, output stored fp32.
- rope without shuffle-DMAs: tb' = q * ssin_permuted elementwise, then 4
  narrow partition-shifted DVE adds implement rotate_half.
- scalar engine runs ONLY the softmax exp (fp32 PSUM -> bf16 SBUF).
- softmax 1/d via vector.reciprocal + ones-stationary broadcast matmul;
  normalize multiplies write ynorm directly (partition-shifted for h1).
- proj PSUM->SBUF staging on gpsimd; x-in/out-DMAs are 4 large
  host-chunked contiguous transfers per 512-token chunk.
- frontend work is interleaved into the attention ki-loop as closures so
  the PE instruction stream stays dense.
"""

from collections import deque

import ml_dtypes
import numpy as np

import concourse.bacc as bacc
import concourse.mybir as mybir
import concourse.tile as tile
from concourse.bass_utils import run_bass_kernel_spmd

F32 = mybir.dt.float32
F32R = mybir.dt.float32r
BF16 = mybir.dt.bfloat16
AF = mybir.ActivationFunctionType

B, T, C = 4, 2048, 1024
N_HEAD, HEAD_DIM = 16, 64
N_CORES = 8
HPC = N_HEAD // N_CORES          # heads per core = 2
HF = HPC * HEAD_DIM              # per-core head features = 128
NT = B * T                       # 8192 tokens
KT = C // 128                    # 8 contraction tiles for qkv proj
QC = 512                         # query-chunk width
NQC = T // QC                    # 4 query chunks per batch
NCHUNK = B * NQC                 # 16 chunks total
ROPE_BASE = 10000.0
SCALE = 1.0 / 8.0                # 1/sqrt(HEAD_DIM)

_PROGRAM = None


def _patch_act_tables():
    import concourse.bacc as _bacc_mod
    from concourse import hw_specs as _hw

    if getattr(_bacc_mod, "_act_tables_patched", False):
        return
    _orig = _hw.get_activation_tables

    def _patched(arch):
        tabs = {k: set(v) for k, v in _orig(arch).items()}
        if "natural_log_exp_and_others" in tabs:
            for name, fns in tabs.items():
                if name != "natural_log_exp_and_others":
                    fns.discard(AF.Exp)
                    fns.discard(AF.Ln)
        return tabs

    _bacc_mod.get_activation_tables = _patched
    _bacc_mod._act_tables_patched = True


def _build_program():
    _patch_act_tables()
    nc = bacc.Bacc(None, target_bir_lowering=False)

    xd = nc.dram_tensor("xd", [NCHUNK * 128, KT * QC], F32R, kind="ExternalInput")
    wq = nc.dram_tensor("wq", [C, HF], F32R, kind="ExternalInput")
    wk = nc.dram_tensor("wk", [C, HF], F32R, kind="ExternalInput")
    wv = nc.dram_tensor("wv", [C, HF], F32R, kind="ExternalInput")
    wpd = nc.dram_tensor("wpd", [HF, C], BF16, kind="ExternalInput")
    identd = nc.dram_tensor("identd", [128, 128], BF16, kind="ExternalInput")
    onesbfd = nc.dram_tensor("onesbfd", [128, 64], BF16, kind="ExternalInput")
    cosd = nc.dram_tensor("cosd", [HF, T], F32, kind="ExternalInput")
    ssinpd = nc.dram_tensor("ssinpd", [HF, T], F32, kind="ExternalInput")
    od = nc.dram_tensor("od", [NCHUNK * 128, KT * QC], F32, kind="ExternalOutput")

    with tile.TileContext(nc) as tc:
        with (
            tc.tile_pool(name="const", bufs=1) as cpool,
            tc.tile_pool(name="sx", bufs=2) as sx,
            tc.tile_pool(name="srope", bufs=2) as srope,
            tc.tile_pool(name="stmp", bufs=2) as stmp,
            tc.tile_pool(name="svt", bufs=2) as svt,
            tc.tile_pool(name="sv", bufs=2) as sv,
            tc.tile_pool(name="spt", bufs=5) as spt,
            tc.tile_pool(name="snrm", bufs=2) as snrm,
            tc.tile_pool(name="sy", bufs=2) as sy,
            tc.tile_pool(name="sst", bufs=3) as sst,
            tc.tile_pool(name="pmm", bufs=2, space="PSUM") as pmm,
            tc.tile_pool(name="psc", bufs=2, space="PSUM") as psc,
            tc.tile_pool(name="py", bufs=2, space="PSUM") as py,
        ):
            # ---- constants (DMA order = first-use order) ----
            wqs = cpool.tile([128, C], F32R, tag="wqs")
            wks = cpool.tile([128, C], F32R, tag="wks")
            wvs = cpool.tile([128, C], F32R, tag="wvs")
            for kt in range(KT):
                nc.gpsimd.dma_start(wqs[:, kt * HF:(kt + 1) * HF], wq[kt * 128:(kt + 1) * 128, :])
                nc.gpsimd.dma_start(wks[:, kt * HF:(kt + 1) * HF], wk[kt * 128:(kt + 1) * 128, :])
                nc.gpsimd.dma_start(wvs[:, kt * HF:(kt + 1) * HF], wv[kt * 128:(kt + 1) * 128, :])
            cost = cpool.tile([128, T], F32, tag="cost")
            nc.gpsimd.dma_start(cost[:], cosd[:])
            ssinpt = cpool.tile([128, T], F32, tag="ssinpt")
            nc.gpsimd.dma_start(ssinpt[:], ssinpd[:])
            ident = cpool.tile([128, 128], BF16, tag="ident")
            nc.gpsimd.dma_start(ident[:], identd[:])
            onesbf = cpool.tile([128, 64], BF16, tag="onesbf")
            nc.gpsimd.dma_start(onesbf[:], onesbfd[:])
            wps = cpool.tile([128, C], BF16, tag="wps")
            nc.gpsimd.dma_start(wps[:], wpd[:])

            state = {}

            def fe_closures(ci):
                """Frontend for chunk ci as a list of closures (interleaved
                into the attention ki loop to keep the PE stream dense)."""
                b, c = divmod(ci, NQC)
                ops = []

                def c_x():
                    if c == 0:
                        state[b] = {
                            "rope_q": srope.tile([128, T], BF16, tag="rope_q", name=f"ropeq{b}"),
                            "rope_k": srope.tile([128, T], BF16, tag="rope_k", name=f"ropek{b}"),
                            "vv": sv.tile([128, (T // 128) * 130], BF16, tag="vv", name=f"vv{b}"),
                        }
                        nc.sync.dma_start(
                            state[b]["vv"][:, 64:(T // 128) * 130:65], onesbf[:, 0:32])
                    st = state[b]
                    st["xs"] = sx.tile([128, KT * QC], F32R, tag="xs", name=f"xs{ci}")
                    for j in range(4):
                        nc.sync.dma_start(
                            st["xs"][:, j * 1024:(j + 1) * 1024],
                            xd[ci * 128:(ci + 1) * 128, j * 1024:(j + 1) * 1024],
                        )
                ops.append(c_x)

                cc = slice(c * QC, (c + 1) * QC)

                def mk_kind(wslab, kind):
                    def c_mm():
                        st = state[b]
                        ps = pmm.tile([128, QC], F32, tag="mm", name=f"qkv{ci}_{kind}")
                        st[f"ps_{kind}"] = ps
                        for kt in range(KT):
                            nc.tensor.matmul(
                                ps[:], wslab[:, kt * HF:(kt + 1) * HF],
                                st["xs"][:, kt * QC:(kt + 1) * QC],
                                start=(kt == 0), stop=(kt == KT - 1),
                            )
                    return c_mm

                def mk_rope(kind):
                    def c_rope():
                        st = state[b]
                        ps = st[f"ps_{kind}"]
                        dst = st["rope_q"] if kind == "q" else st["rope_k"]
                        ta = stmp.tile([128, QC], BF16, tag="ta", name=f"ta{ci}{kind}")
                        tbp = stmp.tile([128, QC], BF16, tag="tb", name=f"tb{ci}{kind}")
                        tb = stmp.tile([128, QC], BF16, tag="tb2", name=f"tc{ci}{kind}")
                        nc.vector.tensor_mul(ta[:], ps[:], cost[:, cc])
                        nc.vector.tensor_mul(tbp[:], ps[:], ssinpt[:, cc])
                        # rotate_half: tb[i] = tbp[swap32(i)], partition-shifted copies
                        for h in range(HPC):
                            hb = h * 64
                            nc.vector.tensor_copy(tb[hb:hb + 32, :], tbp[hb + 32:hb + 64, :])
                            nc.vector.tensor_copy(tb[hb + 32:hb + 64, :], tbp[hb:hb + 32, :])
                        nc.gpsimd.tensor_add(dst[:, cc], ta[:], tb[:])
                    return c_rope

                ops.append(mk_kind(wqs, "q"))
                ops.append(mk_rope("q"))
                ops.append(mk_kind(wks, "k"))
                ops.append(mk_rope("k"))
                ops.append(mk_kind(wvs, "v"))

                def c_v():
                    st = state[b]
                    ps = st["ps_v"]
                    vtc = svt.tile([128, QC], BF16, tag="vtc", name=f"vt{ci}")
                    nc.vector.tensor_copy(vtc[:], ps[:])
                    pst = pmm.tile([128, QC], BF16, tag="mm", name=f"pst{ci}")
                    for t4 in range(4):
                        nc.tensor.transpose(
                            pst[:, t4 * 128:(t4 + 1) * 128],
                            vtc[:, t4 * 128:(t4 + 1) * 128], ident[:])
                    vv = st["vv"]
                    vvr = vv.rearrange("p (a b) -> p a b", b=130).rearrange(
                        "p a (c d) -> p a c d", d=65)
                    pstr = pst.rearrange("p (a b) -> p a b", b=128).rearrange(
                        "p a (c d) -> p a c d", d=64)
                    for half in range(2):
                        ki0 = 4 * c + 2 * half
                        nc.vector.tensor_copy(
                            vvr[:, ki0:ki0 + 2, 0:2, 0:64],
                            pstr[:, 2 * half:2 * half + 2, :, :],
                        )
                ops.append(c_v)
                return ops

            def make_tail(ci, y_aug):
                """Normalize + projection closures for chunk ci. The first
                BARRIER closures read y_aug and must all be emitted before the
                NEXT chunk's first PV (its y-slot reuse)."""
                ctx = {}

                def t_rec():
                    rec = snrm.tile([128, 2 * QC], BF16, tag="rec", name=f"rec{ci}")
                    lnt = snrm.tile([128, 2 * QC], F32R, tag="lnt", name=f"lnt{ci}")
                    ctx["rec"] = rec
                    for h in range(HPC):
                        nc.scalar.activation(
                            lnt[64:65, h * QC:(h + 1) * QC], y_aug[h][64:65, :], AF.Ln)
                        nc.scalar.activation(
                            rec[64:65, h * QC:(h + 1) * QC],
                            lnt[64:65, h * QC:(h + 1) * QC], AF.Exp, scale=-1.0)

                def t_bc():
                    rec = ctx["rec"]
                    bcs = snrm.tile([128, 2 * QC], F32R, tag="bcs", name=f"bcs{ci}")
                    ctx["bcs"] = bcs
                    bcp = []
                    for h in range(HPC):
                        bc = pmm.tile([128, QC], F32, tag="mm", name=f"bc{ci}_{h}")
                        nc.tensor.matmul(
                            bc[0:64, :], onesbf[64:65, 0:64],
                            rec[64:65, h * QC:(h + 1) * QC],
                            start=True, stop=True,
                        )
                        bcp.append(bc)
                    for h in range(HPC):
                        nc.vector.tensor_copy(
                            bcs[0:64, h * QC:(h + 1) * QC], bcp[h][0:64, :])

                def t_ynorm():
                    bcs = ctx["bcs"]
                    ynorm = sy.tile([128, QC], BF16, tag="ynorm", name=f"yn{ci}")
                    ctx["ynorm"] = ynorm
                    for h in range(HPC):
                        nc.vector.tensor_mul(
                            ynorm[h * 64:(h + 1) * 64, :], y_aug[h][0:64, :],
                            bcs[0:64, h * QC:(h + 1) * QC])

                def mk_proj(j):
                    def t_proj():
                        if j == 0:
                            ctx["stg"] = sst.tile(
                                [128, KT * QC], F32, tag="st", name=f"stg{ci}")
                        stg = ctx["stg"]
                        ynorm = ctx["ynorm"]
                        op = psc.tile([128, 2 * QC], F32, tag="sc", name=f"op{ci}_{j}")
                        for s2 in range(2):
                            of = 2 * j + s2
                            nc.tensor.matmul(
                                op[:, s2 * QC:(s2 + 1) * QC],
                                wps[:, of * 128:(of + 1) * 128], ynorm[:],
                                start=True, stop=True,
                            )
                        if j % 2 == 0:
                            nc.vector.tensor_copy(stg[:, j * 1024:(j + 1) * 1024], op[:])
                        else:
                            nc.scalar.copy(stg[:, j * 1024:(j + 1) * 1024], op[:])
                        nc.sync.dma_start(
                            od[ci * 128:(ci + 1) * 128, j * 1024:(j + 1) * 1024],
                            stg[:, j * 1024:(j + 1) * 1024],
                        )
                    return t_proj

                return deque([t_rec, t_bc, t_ynorm] + [mk_proj(j) for j in range(4)])

            TAIL_BARRIER = 3  # t_rec, t_bc, t_ynorm read the previous y tiles

            def attention_chunk(ci, fe_q, tail_q):
                b, qc = divmod(ci, NQC)
                st = state[b]
                rope_q, rope_k, vv = st["rope_q"], st["rope_k"], st["vv"]
                qs = qc * QC
                nki = (qs + QC) // 128
                y_aug = [py.tile([65, QC], F32, tag="y", name=f"y{ci}_{h}")
                         for h in range(HPC)]

                def emit_pv(pt, ki, c0):
                    for h in range(HPC):
                        nc.tensor.matmul(
                            y_aug[h][0:65, c0:QC],
                            vv[:, ki * 130 + 65 * h: ki * 130 + 65 * h + 65],
                            pt[:, h * QC + c0: (h + 1) * QC],
                            start=(ki == 0),
                            stop=(ki == nki - 1),
                        )

                barrier = TAIL_BARRIER if tail_q else 0
                popped = 0
                pvq = []
                for ki in range(nki):
                    c0 = max(0, ki * 128 - qs)
                    diag = ki * 128 >= qs
                    sc = psc.tile([128, 2 * QC], F32, tag="sc", name=f"sc{ci}_{ki}")
                    for h in range(HPC):
                        hb = h * 64
                        nc.tensor.matmul(
                            sc[:, h * QC + c0:(h + 1) * QC],
                            rope_k[hb:hb + 64, ki * 128:(ki + 1) * 128],
                            rope_q[hb:hb + 64, qs + c0:qs + QC],
                            start=True, stop=True,
                        )
                    pt = spt.tile([128, 2 * QC], BF16, tag="pt", name=f"pt{ci}_{ki}")
                    if c0 == 0:
                        nc.scalar.activation(
                            pt[:, 0:2 * QC], sc[:, 0:2 * QC], AF.Exp, scale=SCALE,
                        )
                    else:
                        for h in range(HPC):
                            nc.scalar.activation(
                                pt[:, h * QC + c0:(h + 1) * QC],
                                sc[:, h * QC + c0:(h + 1) * QC], AF.Exp, scale=SCALE,
                            )
                    if diag:
                        for h in range(HPC):
                            nc.gpsimd.affine_select(
                                pt[:, h * QC + c0: h * QC + c0 + 128],
                                pt[:, h * QC + c0: h * QC + c0 + 128],
                                pattern=[[1, 128]],
                                compare_op=mybir.AluOpType.is_ge,
                                fill=0.0, base=0, channel_multiplier=-1,
                            )
                    # fill the exp(ki) latency: prev-chunk tail first, then fe
                    if tail_q:
                        tail_q.popleft()()
                        popped += 1
                    elif fe_q and len(fe_q) > 2:
                        fe_q.popleft()()
                    pvq.append((pt, ki, c0))
                    if popped >= barrier:
                        while len(pvq) > 1:
                            emit_pv(*pvq.pop(0))
                for args in pvq:
                    emit_pv(*args)
                while tail_q:
                    tail_q.popleft()()
                while fe_q:
                    fe_q.popleft()()
                if qc == NQC - 1:
                    state.pop(b)
                return make_tail(ci, y_aug)

            # ---- software pipeline over chunks ----
            fe_q = deque()
            for op in fe_closures(0):
                op()
            for op in fe_closures(1):
                op()
            tail_q = deque()
            for i in range(NCHUNK):
                if i + 2 < NCHUNK:
                    fe_q.extend(fe_closures(i + 2))
                tail_q = attention_chunk(i, fe_q, tail_q)
            while tail_q:
                tail_q.popleft()()
    nc.finalize()
    return nc


def _program():
    global _PROGRAM
    if _PROGRAM is None:
        _PROGRAM = _build_program()
    return _PROGRAM


def _rope_tables():
    inv_freq = 1.0 / (ROPE_BASE ** (np.arange(0, HEAD_DIM, 2, dtype=np.float32) / HEAD_DIM))
    t = np.arange(T, dtype=np.float32)
    freqs = np.outer(t, inv_freq).astype(np.float32)        # [T, 32]
    emb = np.concatenate([freqs, freqs], axis=1)            # [T, 64]
    cos = np.cos(emb).astype(np.float32)                    # [T, 64]
    sin_fr = np.sin(freqs).astype(np.float32)               # [T, 32]
    # ssin_permuted: [:32] = +sin_fr, [32:] = -sin_fr
    ssinp = np.concatenate([sin_fr, -sin_fr], axis=1)       # [T, 64]
    cosT = np.ascontiguousarray(cos.T)                      # [64, T]
    ssinpT = np.ascontiguousarray(ssinp.T)
    cos2 = np.concatenate([cosT] * HPC, axis=0)             # [128, T]
    ssinp2 = np.concatenate([ssinpT] * HPC, axis=0)
    return cos2, ssinp2


def _prep_in_maps(x, W_attn, W_proj):
    x = np.asarray(x, dtype=np.float32)
    W_attn = np.asarray(W_attn, dtype=np.float32)
    W_proj = np.asarray(W_proj, dtype=np.float32)
    # xd[(b*NQC+qc)*128 + p, kt*QC + j] = x[b, qc*QC+j, kt*128+p]
    xd = np.ascontiguousarray(
        x.reshape(B, NQC, QC, KT, 128).transpose(0, 1, 4, 3, 2)
        .reshape(NCHUNK * 128, KT * QC))
    cos2, ssinp2 = _rope_tables()
    in_maps = []
    for i in range(N_CORES):
        cs = i * HF
        in_maps.append({
            "xd": xd,
            "wq": np.ascontiguousarray(W_attn[:, cs:cs + HF]),
            "wk": np.ascontiguousarray(W_attn[:, C + cs:C + cs + HF]),
            "wv": np.ascontiguousarray(W_attn[:, 2 * C + cs:2 * C + cs + HF]),
            "wpd": np.ascontiguousarray(W_proj[cs:cs + HF, :]).astype(ml_dtypes.bfloat16),
            "identd": np.eye(128, dtype=ml_dtypes.bfloat16),
            "onesbfd": np.ones((128, 64), dtype=ml_dtypes.bfloat16),
            "cosd": cos2,
            "ssinpd": ssinp2,
        })
    return in_maps


def _gather(res):
    acc = np.zeros((NCHUNK * 128, KT * QC), dtype=np.float32)
    for r in res.results:
        acc += r["od"]
    # od[(b*NQC+qc)*128 + p, of*QC + j] = out[b, qc*QC+j, of*128+p]
    return np.ascontiguousarray(
        acc.reshape(B, NQC, 128, KT, QC).transpose(0, 1, 4, 3, 2)
        .reshape(B, T, C))


def _run(in_maps, trace=False, **kwargs):
    return run_bass_kernel_spmd(
        _program(), in_maps, core_ids=list(range(N_CORES)), trace=trace, **kwargs
    )


def kernel(x, W_attn, W_proj):
    in_maps = _prep_in_maps(x, W_attn, W_proj)
    res = _run(in_maps)
    return _gather(res)
